# revision 11
# baseline (speedup 1.0000x reference)
"""Trainium2 Bass kernel for nn_ActorAction (moe_routing).

Computation (see reference):
  option_embed = embed_table[option]              [B, 64]
  all_state    = concat([state, option_embed])    [B, 576]
  cls_X = MLP_relu(all_state; Wx1,bx1,Wx2,bx2)    [B, 256]
  cls_Y = MLP_relu(all_state; Wy1,by1,Wy2,by2)    [B, 256]
  out_X = cls_X @ noise_lib_X                     [B, 256]
  out_Y[b] = cls_Y[b] @ noise_lib_Y[option[b]]    [B, 256]

Strategy: data-parallel over batch across 8 cores. Host sorts samples by
class and deals them round-robin so that every core has the IDENTICAL
per-class slot structure (the Bass program is shared SPMD). All device
matmuls run in feature-major ("transposed") orientation so weights are
the stationary operand and the per-class routing matmuls take cheap
free-dim slices of cls_Y as stationary weights. Routing outputs land in
32-aligned PSUM col-groups (4 class slots per PSUM tile -> concurrent
PE tiles), then coalesced DMAs out. Input DMAs are split and ordered by
first consumer so the PE starts as early as possible; a stream of tiny
dummy matmuls warms the PE (HAM un-throttle) before real data lands.
"""
import os
from contextlib import ExitStack

import numpy as np
import ml_dtypes

import concourse.bacc as bacc
import concourse.mybir as mybir
import concourse.tile as tile
from concourse.bass_utils import run_bass_kernel_spmd
from concourse.tile_rust import add_dep_helper

F32 = mybir.dt.float32
F32R = mybir.dt.float32r
BF16 = mybir.dt.bfloat16
AFT = mybir.ActivationFunctionType

# problem dims (hardcoded per spec)
B, FEAT, EMB, HID, NCLS = 4096, 512, 64, 1024, 64
LIB = 256          # LIB_X == LIB_Y
OUTJ = 256
NCORES = 8
D_IN = FEAT + EMB          # 576
KO1 = 5                    # ceil(576/128) K-blocks for layer 1
D_PAD = KO1 * 128          # 640
KO2 = HID // 128           # 8
NY_GROUPS = 4
CLS_PER_GROUP = NCLS // NY_GROUPS  # 16
N_WARMUP = 30              # dummy matmuls to keep PE busy/warm during loads

_DT_MAP = {"f32": F32, "f32r": F32R, "bf16": BF16}
_NP_MAP = {"f32": np.float32, "f32r": np.float32, "bf16": ml_dtypes.bfloat16}
DT_A_NAME = os.environ.get("KDT_A", "bf16")    # MLP weights/acts + NX path
DT_NY_NAME = os.environ.get("KDT_NY", "bf16")  # noise_lib_Y + cls_Y path


def _round_up(a, b):
    return (a + b - 1) // b * b


def _plan(option):
    opt = np.asarray(option).astype(np.int64).ravel()
    assert opt.shape[0] == B
    g = np.bincount(opt, minlength=NCLS)
    u = (g + NCORES - 1) // NCORES           # per-core capacity per class
    # slots: (class m, class_pos_offset j0, compact_start s, rows r, out_slot k)
    slots = []
    s = 0
    k = 0
    for m in range(NCLS):
        um = int(u[m])
        j0 = 0
        while um > 0:
            r = min(32, um)
            slots.append((m, j0, s, r, k))
            s += r
            k += 1
            um -= r
            j0 += r
    SU = s
    nslots = k
    nplanes = (nslots + 3) // 4
    SU_pad = max(_round_up(SU, 32), 128)
    if SU_pad <= 512:
        chunks = [(0, SU_pad)]
    else:
        half = _round_up((SU_pad + 1) // 2, 32)
        chunks = [(0, half), (half, SU_pad)]

    # deal samples: class m's j-th sample (sorted) -> core j%8, pos j//8
    order = np.argsort(opt, kind="stable")
    starts = np.concatenate([[0], np.cumsum(g)])
    core_of = np.empty(B, np.int64)
    col_of = np.empty(B, np.int64)
    outrow_of = np.empty(B, np.int64)
    for m, j0, s0, r, kk in slots:
        gm = int(g[m])
        if gm == 0:
            continue
        idx = order[starts[m]:starts[m] + gm]
        j = np.arange(gm)
        pos = j // NCORES
        mask = (pos >= j0) & (pos < j0 + r)
        if not mask.any():
            continue
        core_of[idx[mask]] = j[mask] % NCORES
        col_of[idx[mask]] = s0 + pos[mask] - j0
        outrow_of[idx[mask]] = 128 * (kk // 4) + 32 * (kk % 4) + pos[mask] - j0

    # plane -> NY group of its last slot (for per-group output DMAs)
    plane_group = []
    for qi in range(nplanes):
        quad = slots[4 * qi:4 * qi + 4]
        plane_group.append(quad[-1][0] // CLS_PER_GROUP)

    return dict(opt=opt, slots=slots, SU=SU, SU_pad=SU_pad, chunks=chunks,
                nslots=nslots, nplanes=nplanes, core_of=core_of,
                col_of=col_of, outrow_of=outrow_of, plane_group=plane_group)


_NC_CACHE = {}


def _build_nc(plan):
    DT_A = _DT_MAP[DT_A_NAME]
    DT_NY = _DT_MAP[DT_NY_NAME]
    SU_pad = plan["SU_pad"]
    chunks = plan["chunks"]
    slots = plan["slots"]
    nplanes = plan["nplanes"]
    plane_group = plan["plane_group"]

    key = (tuple(slots), SU_pad, DT_A_NAME, DT_NY_NAME)
    if key in _NC_CACHE:
        return _NC_CACHE[key]

    c_xt = KO1 * SU_pad
    c_w1 = KO1 * HID
    c_w2 = KO2 * LIB
    c_nx = 2 * OUTJ
    c_bx = c_w1 + c_w2 + c_nx          # X-branch blob columns
    NY_COLS = CLS_PER_GROUP * 2 * OUTJ  # per group

    ch0 = chunks[0][1]
    c_xt_a = KO1 * ch0
    c_xt_b = c_xt - c_xt_a
    c_mo = KO1 * 128           # one mo-block of W1

    nc = bacc.Bacc()
    xt_a_d = nc.dram_tensor("xt_a", [128, c_xt_a], DT_A, kind="ExternalInput")
    xt_b_d = (nc.dram_tensor("xt_b", [128, c_xt_b], DT_A, kind="ExternalInput")
              if c_xt_b else None)
    w1y_d = nc.dram_tensor("w1y", [128, c_w1], DT_A, kind="ExternalInput")
    w2y_d = nc.dram_tensor("w2y", [128, c_w2], DT_A, kind="ExternalInput")
    blobx_d = nc.dram_tensor("blobx", [128, c_bx], DT_A, kind="ExternalInput")
    bias_d = nc.dram_tensor("bias", [128, 20], F32, kind="ExternalInput")
    ny_d = nc.dram_tensor("ny", [NY_GROUPS, 128, NY_COLS], DT_NY,
                          kind="ExternalInput")
    outx_d = nc.dram_tensor("outx", [2 * 128, SU_pad], F32, kind="ExternalOutput")
    outy_d = nc.dram_tensor("outy", [nplanes * 128, OUTJ], F32,
                            kind="ExternalOutput")

    with tile.TileContext(nc) as tc, ExitStack() as ctx:
        const = ctx.enter_context(tc.tile_pool(name="const", bufs=1))
        act = ctx.enter_context(tc.tile_pool(name="act", bufs=1))
        hpool = ctx.enter_context(tc.tile_pool(name="hpool", bufs=1))
        mlp_ps = ctx.enter_context(tc.tile_pool(name="mlp_ps", bufs=4, space="PSUM"))
        rt_ps = ctx.enter_context(tc.tile_pool(name="rt_ps", bufs=3, space="PSUM"))
        wu_ps = ctx.enter_context(tc.tile_pool(name="wu_ps", bufs=1, space="PSUM"))

        # input DMAs: many small parallel streams for the Y-branch inputs
        # (fair-share ring arbitration finishes small streams early), NY
        # gated behind them via explicit deps.
        bias_sb = const.tile([128, 20], F32)
        nc.sync.dma_start(bias_sb[:], bias_d[:])
        pre_ny = []
        xt_a_sb = const.tile([128, c_xt_a], DT_A)
        n_sp = 4
        step = _round_up(c_xt_a // n_sp, KO1)
        for i in range(n_sp):
            lo, hi = i * step, min((i + 1) * step, c_xt_a)
            if lo < hi:
                pre_ny.append(nc.sync.dma_start(xt_a_sb[:, lo:hi], xt_a_d[:, lo:hi]))
        # per-mo W1 tiles (separate tiles => precise per-block deps)
        w1_tiles = {"y": [], "x": []}
        for mo in range(KO2):
            t = const.tile([128, c_mo], DT_A, tag=f"w1y{mo}", name=f"w1y{mo}")
            pre_ny.append(nc.sync.dma_start(
                t[:], w1y_d[:, mo * c_mo:(mo + 1) * c_mo]))
            w1_tiles["y"].append(t.rearrange("p (ko m) -> p ko m", ko=KO1))
        if c_xt_b:
            xt_b_sb = const.tile([128, c_xt_b], DT_A)
            pre_ny.append(nc.sync.dma_start(xt_b_sb[:], xt_b_d[:]))
        w2y_sb = const.tile([128, c_w2], DT_A)
        pre_ny.append(nc.sync.dma_start(w2y_sb[:], w2y_d[:]))
        for mo in range(KO2):
            t = const.tile([128, c_mo], DT_A, tag=f"w1x{mo}", name=f"w1x{mo}")
            nc.sync.dma_start(t[:], blobx_d[:, mo * c_mo:(mo + 1) * c_mo])
            w1_tiles["x"].append(t.rearrange("p (ko m) -> p ko m", ko=KO1))
        w2x_sb = const.tile([128, c_w2], DT_A)
        nc.sync.dma_start(w2x_sb[:], blobx_d[:, c_w1:c_w1 + c_w2])
        nx_sb = const.tile([128, c_nx], DT_A)
        nc.sync.dma_start(nx_sb[:], blobx_d[:, c_w1 + c_w2:])
        ny_sb = []
        for gi in range(NY_GROUPS):
            t = const.tile([128, NY_COLS], DT_NY, tag=f"ny{gi}", name=f"ny{gi}")
            d = nc.sync.dma_start(t[:], ny_d[gi])
            for b in pre_ny:
                add_dep_helper(d.ins, b.ins, sync=True,
                               reason="ny after y-branch inputs")
            ny_sb.append(t.rearrange("p (m ko j) -> p m ko j", m=CLS_PER_GROUP, ko=2))

        # PE warmup: tiny dummy matmuls on the bias tile keep the PE busy
        # (HAM un-throttles) while the real inputs stream in.
        bias_bf = const.tile([128, 40], BF16)
        nc.vector.tensor_copy(bias_bf[:], bias_sb.bitcast(mybir.dt.uint16)[:])
        wups = wu_ps.tile([40, 40], F32)
        for _ in range(N_WARMUP):
            nc.tensor.matmul(wups[:], lhsT=bias_bf[:, :40], rhs=bias_bf[:],
                             start=True, stop=True)

        xt_vc = [xt_a_sb.rearrange("p (ko b) -> p ko b", ko=KO1)]
        if c_xt_b:
            xt_vc.append(xt_b_sb.rearrange("p (ko b) -> p ko b", ko=KO1))

        def w1v(br, mo, ko, msl):
            return w1_tiles[br][mo][:, ko, msl]

        w2_v = {"y": w2y_sb.rearrange("p (ko m) -> p ko m", ko=KO2),
                "x": w2x_sb.rearrange("p (ko m) -> p ko m", ko=KO2)}
        nx_v = nx_sb.rearrange("p (ko j) -> p ko j", ko=2)
        # bias cols: b1y[0:8] b2y[8:10] b1x[10:18] b2x[18:20]
        bcol = {"y": (0, 8), "x": (10, 18)}

        cls_sb = {}
        CH_MAX = max(c1 - c0 for c0, c1 in chunks)
        for br in ("y", "x"):
            dt_cls = DT_NY if br == "y" else DT_A
            cls_sb[br] = act.tile([128, 2, SU_pad], dt_cls, tag=f"cls{br}",
                                  name=f"cls{br}")
            h_sb = hpool.tile([128, KO2, SU_pad], DT_A, tag="h", name=f"h_{br}")
            b1o, b2o = bcol[br]
            for ci, (c0, c1) in enumerate(chunks):
                cw = c1 - c0
                for mo in range(KO2):
                    ps = mlp_ps.tile([128, CH_MAX], F32, tag="mlp",
                                     name="mlp_ps_t")[:, :cw]
                    for ko in range(KO1):
                        nc.tensor.matmul(
                            ps, lhsT=w1v(br, mo, ko, slice(0, 128)),
                            rhs=xt_vc[ci][:, ko, :cw],
                            start=(ko == 0), stop=(ko == KO1 - 1))
                    nc.scalar.activation(h_sb[:, mo, c0:c1], ps, AFT.Relu,
                                         bias=bias_sb[:, b1o + mo:b1o + mo + 1])
            for c0, c1 in chunks:
                cw = c1 - c0
                for jo in range(2):
                    ps = mlp_ps.tile([128, CH_MAX], F32, tag="mlp",
                                     name="mlp_ps_t")[:, :cw]
                    for ko in range(KO2):
                        nc.tensor.matmul(
                            ps, lhsT=w2_v[br][:, ko, jo * 128:(jo + 1) * 128],
                            rhs=h_sb[:, ko, c0:c1],
                            start=(ko == 0), stop=(ko == KO2 - 1))
                    nc.scalar.activation(cls_sb[br][:, jo, c0:c1], ps, AFT.Identity,
                                         bias=bias_sb[:, b2o + jo:b2o + jo + 1])

        # X output: outX.T[j, b] = sum_i NX[i, j] * clsX.T[i, b]
        outxT = act.tile([128, 2, SU_pad], F32, tag="outxT")
        for c0, c1 in chunks:
            cw = c1 - c0
            for jo in range(2):
                ps = mlp_ps.tile([128, CH_MAX], F32, tag="mlp",
                                 name="mlp_ps_t")[:, :cw]
                for ko in range(2):
                    nc.tensor.matmul(ps, lhsT=nx_v[:, ko, jo * 128:(jo + 1) * 128],
                                     rhs=cls_sb["x"][:, ko, c0:c1],
                                     start=(ko == 0), stop=(ko == 1))
                nc.scalar.copy(outxT[:, jo, c0:c1], ps)
        nc.sync.dma_start(outx_d.rearrange("(jo p) b -> p jo b", p=128), outxT[:])

        # routing: per slot (class m, rows r at compact col s), out_Y.seg =
        # clsY.T[:, s:s+r].T @ NY[m]; psum col-group 32*(k%4), plane k//4
        outy_sb = act.tile([128, nplanes, OUTJ], F32, tag="outy")
        outy_dv = outy_d.rearrange("(o p) j -> p o j", p=128)
        clsy = cls_sb["y"]
        for qi in range(nplanes):
            quad = slots[4 * qi:4 * qi + 4]
            ps = rt_ps.tile([128, OUTJ], F32, tag="rt", name="rt_ps_t")
            for si, (m, _j0, s, r, _k) in enumerate(quad):
                gi, mi = m // CLS_PER_GROUP, m % CLS_PER_GROUP
                for ko in range(2):
                    nc.tensor.matmul(ps[32 * si:32 * si + r, :],
                                     lhsT=clsy[:, ko, s:s + r],
                                     rhs=ny_sb[gi][:, mi, ko, :],
                                     start=(ko == 0), stop=(ko == 1),
                                     tile_position=(0, 32 * si))
            nc.vector.tensor_copy(outy_sb[:, qi, :], ps[:])
            last_of_group = (qi + 1 == nplanes or
                             plane_group[qi + 1] != plane_group[qi])
            if last_of_group:
                p0 = next(i for i in range(nplanes)
                          if plane_group[i] == plane_group[qi])
                nc.sync.dma_start(outy_dv[:, p0:qi + 1, :],
                                  outy_sb[:, p0:qi + 1, :])

    nc.compile()
    _NC_CACHE[key] = nc
    return nc


def _prepare_inputs(plan, state, option, embed_table, Wx1, bx1, Wx2, bx2,
                    Wy1, by1, Wy2, by2, noise_lib_X, noise_lib_Y):
    np_a = _NP_MAP[DT_A_NAME]
    np_ny = _NP_MAP[DT_NY_NAME]
    SU_pad = plan["SU_pad"]
    opt = plan["opt"]
    core_of, col_of = plan["core_of"], plan["col_of"]

    state = np.asarray(state, np.float32)
    embed_table = np.asarray(embed_table, np.float32)

    # per-core feature-major inputs
    Xall = np.zeros((NCORES, SU_pad, D_PAD), np.float32)
    Xall[core_of, col_of, :FEAT] = state
    Xall[core_of, col_of, FEAT:D_IN] = embed_table[opt]
    # [NCORES, 128, KO1, SU_pad]
    xt = Xall.transpose(0, 2, 1).reshape(NCORES, KO1, 128, SU_pad) \
        .transpose(0, 2, 1, 3).astype(np_a)
    ch0 = plan["chunks"][0][1]
    xt_a = np.ascontiguousarray(xt[:, :, :, :ch0]).reshape(NCORES, 128, -1)
    xt_b = np.ascontiguousarray(xt[:, :, :, ch0:]).reshape(NCORES, 128, -1)

    def pack_w1(w):
        # mo-major: [128p, mo, ko, 128] flattened
        wp = np.zeros((D_PAD, HID), np.float32)
        wp[:D_IN] = np.asarray(w, np.float32)
        return wp.reshape(KO1, 128, KO2, 128).transpose(1, 2, 0, 3) \
            .reshape(128, KO1 * HID)

    def pack_w2(w):
        return np.asarray(w, np.float32).reshape(KO2, 128, LIB) \
            .transpose(1, 0, 2).reshape(128, KO2 * LIB)

    nx = np.asarray(noise_lib_X, np.float32).reshape(2, 128, OUTJ) \
        .transpose(1, 0, 2).reshape(128, 2 * OUTJ)
    w1y = np.ascontiguousarray(pack_w1(Wy1).astype(np_a))
    w2y = np.ascontiguousarray(pack_w2(Wy2).astype(np_a))
    blobx = np.ascontiguousarray(np.concatenate(
        [pack_w1(Wx1), pack_w2(Wx2), nx], axis=1).astype(np_a))

    bias = np.zeros((128, 20), np.float32)
    bias[:, 0:8] = np.asarray(by1, np.float32).reshape(8, 128).T
    bias[:, 8:10] = np.asarray(by2, np.float32).reshape(2, 128).T
    bias[:, 10:18] = np.asarray(bx1, np.float32).reshape(8, 128).T
    bias[:, 18:20] = np.asarray(bx2, np.float32).reshape(2, 128).T

    ny = np.ascontiguousarray(
        np.asarray(noise_lib_Y, np.float32)
        .reshape(NY_GROUPS, CLS_PER_GROUP, 2, 128, OUTJ)
        .transpose(0, 3, 1, 2, 4)
        .reshape(NY_GROUPS, 128, CLS_PER_GROUP * 2 * OUTJ).astype(np_ny))

    in_maps = []
    for c in range(NCORES):
        m = {"xt_a": xt_a[c], "w1y": w1y, "w2y": w2y,
             "blobx": blobx, "bias": bias, "ny": ny}
        if xt_b.shape[-1]:
            m["xt_b"] = xt_b[c]
        in_maps.append(m)
    return in_maps


def _gather_outputs(plan, results):
    core_of, col_of, outrow_of = (plan["core_of"], plan["col_of"],
                                  plan["outrow_of"])
    ox = np.stack([r["outx"] for r in results])   # [8, 256, SU_pad]
    oy = np.stack([r["outy"] for r in results])   # [8, nplanes*128, OUTJ]
    gx = ox.transpose(0, 2, 1)[core_of, col_of].astype(np.float32)
    gy = oy[core_of, outrow_of].astype(np.float32)
    return gx, gy


def _run(inputs, trace=False):
    plan = _plan(inputs["option"])
    nc = _build_nc(plan)
    in_maps = _prepare_inputs(plan, **inputs)
    res = run_bass_kernel_spmd(nc, in_maps, core_ids=list(range(NCORES)),
                               trace=trace)
    gx, gy = _gather_outputs(plan, res.results)
    return (gx, gy), res


def kernel(**inputs):
    (gx, gy), _ = _run(inputs, trace=False)
    return gx, gy


# revision 20
# speedup vs baseline: 1.1516x; 1.1516x over previous
"""Trainium2 Bass kernel for nn_ActorAction (moe_routing).

Computation (see reference):
  option_embed = embed_table[option]              [B, 64]
  all_state    = concat([state, option_embed])    [B, 576]
  cls_X = MLP_relu(all_state; Wx1,bx1,Wx2,bx2)    [B, 256]
  cls_Y = MLP_relu(all_state; Wy1,by1,Wy2,by2)    [B, 256]
  out_X = cls_X @ noise_lib_X                     [B, 256]
  out_Y[b] = cls_Y[b] @ noise_lib_Y[option[b]]    [B, 256]

Strategy: data-parallel over batch across 8 cores. Host sorts samples by
class and deals them round-robin so that every core has the IDENTICAL
per-class slot structure (the Bass program is shared SPMD). All device
matmuls run in feature-major ("transposed") orientation so weights are
the stationary operand and the per-class routing matmuls take cheap
free-dim slices of cls_Y as stationary weights. Routing outputs land in
32-aligned PSUM col-groups (4 class slots per PSUM tile -> concurrent
PE tiles), then coalesced DMAs out. Input DMAs are split and ordered by
first consumer so the PE starts as early as possible; a stream of tiny
dummy matmuls warms the PE (HAM un-throttle) before real data lands.
"""
import os
from contextlib import ExitStack

import numpy as np
import ml_dtypes

import concourse.bacc as bacc
import concourse.mybir as mybir
import concourse.tile as tile
from concourse.bass_utils import run_bass_kernel_spmd

F32 = mybir.dt.float32
F32R = mybir.dt.float32r
BF16 = mybir.dt.bfloat16
AFT = mybir.ActivationFunctionType

# problem dims (hardcoded per spec)
B, FEAT, EMB, HID, NCLS = 4096, 512, 64, 1024, 64
LIB = 256          # LIB_X == LIB_Y
OUTJ = 256
NCORES = 8
D_IN = FEAT + EMB          # 576
KO1 = 5                    # ceil(576/128) K-blocks for layer 1
D_PAD = KO1 * 128          # 640
KO2 = HID // 128           # 8
NY_GROUPS = 4
CLS_PER_GROUP = NCLS // NY_GROUPS  # 16
N_WARMUP = 110             # dummy matmuls to keep PE busy/warm during loads

_DT_MAP = {"f32": F32, "f32r": F32R, "bf16": BF16}
_NP_MAP = {"f32": np.float32, "f32r": np.float32, "bf16": ml_dtypes.bfloat16}
DT_A_NAME = os.environ.get("KDT_A", "bf16")    # MLP weights/acts + NX path
DT_NY_NAME = os.environ.get("KDT_NY", "bf16")  # noise_lib_Y + cls_Y path


def _round_up(a, b):
    return (a + b - 1) // b * b


def _plan(option):
    opt = np.asarray(option).astype(np.int64).ravel()
    assert opt.shape[0] == B
    g = np.bincount(opt, minlength=NCLS)
    u = (g + NCORES - 1) // NCORES           # per-core capacity per class
    # slots: (class m, class_pos_offset j0, compact_start s, rows r, out_slot k)
    slots = []
    s = 0
    k = 0
    for m in range(NCLS):
        um = int(u[m])
        j0 = 0
        while um > 0:
            r = min(32, um)
            slots.append((m, j0, s, r, k))
            s += r
            k += 1
            um -= r
            j0 += r
    SU = s
    nslots = k
    nplanes = (nslots + 3) // 4
    SU_pad = max(_round_up(SU, 32), 128)
    if SU_pad <= 512:
        chunks = [(0, SU_pad)]
    else:
        half = _round_up((SU_pad + 1) // 2, 32)
        chunks = [(0, half), (half, SU_pad)]

    # deal samples: class m's j-th sample (sorted) -> core j%8, pos j//8
    order = np.argsort(opt, kind="stable")
    starts = np.concatenate([[0], np.cumsum(g)])
    core_of = np.empty(B, np.int64)
    col_of = np.empty(B, np.int64)
    outrow_of = np.empty(B, np.int64)
    for m, j0, s0, r, kk in slots:
        gm = int(g[m])
        if gm == 0:
            continue
        idx = order[starts[m]:starts[m] + gm]
        j = np.arange(gm)
        pos = j // NCORES
        mask = (pos >= j0) & (pos < j0 + r)
        if not mask.any():
            continue
        core_of[idx[mask]] = j[mask] % NCORES
        col_of[idx[mask]] = s0 + pos[mask] - j0
        outrow_of[idx[mask]] = 128 * (kk // 4) + 32 * (kk % 4) + pos[mask] - j0

    # plane -> NY group of its last slot (for per-group output DMAs)
    plane_group = []
    for qi in range(nplanes):
        quad = slots[4 * qi:4 * qi + 4]
        plane_group.append(quad[-1][0] // CLS_PER_GROUP)

    return dict(opt=opt, slots=slots, SU=SU, SU_pad=SU_pad, chunks=chunks,
                nslots=nslots, nplanes=nplanes, core_of=core_of,
                col_of=col_of, outrow_of=outrow_of, plane_group=plane_group)


_NC_CACHE = {}


def _build_nc(plan):
    DT_A = _DT_MAP[DT_A_NAME]
    DT_NY = _DT_MAP[DT_NY_NAME]
    SU_pad = plan["SU_pad"]
    chunks = plan["chunks"]
    slots = plan["slots"]
    nplanes = plan["nplanes"]
    plane_group = plan["plane_group"]

    key = (tuple(slots), SU_pad, DT_A_NAME, DT_NY_NAME)
    if key in _NC_CACHE:
        return _NC_CACHE[key]

    c_xt = KO1 * SU_pad
    c_w1 = KO1 * HID
    c_w2 = KO2 * LIB
    c_nx = 2 * OUTJ
    c_bx = c_w1 + c_w2 + c_nx          # X-branch blob columns
    NY_COLS = CLS_PER_GROUP * 2 * OUTJ  # per group

    ch0 = chunks[0][1]
    c_xt_a = KO1 * ch0
    c_xt_b = c_xt - c_xt_a
    c_mo = KO1 * 128           # one mo-block of W1

    nc = bacc.Bacc()
    xt_a_d = nc.dram_tensor("xt_a", [128, c_xt_a], DT_A, kind="ExternalInput")
    xt_b_d = (nc.dram_tensor("xt_b", [128, c_xt_b], DT_A, kind="ExternalInput")
              if c_xt_b else None)
    w1y_d = nc.dram_tensor("w1y", [128, c_w1], DT_A, kind="ExternalInput")
    # w1y piece boundaries (in mo blocks): mo0 | mo1-3 | mo4-7
    w1y_pieces = [(0, 1), (1, 4), (4, KO2)]
    w2y_d = nc.dram_tensor("w2y", [128, c_w2], DT_A, kind="ExternalInput")
    blobx_d = nc.dram_tensor("blobx", [128, c_bx], DT_A, kind="ExternalInput")
    bias_d = nc.dram_tensor("bias", [128, 20], F32, kind="ExternalInput")
    ny_d = nc.dram_tensor("ny", [NY_GROUPS, 128, NY_COLS], DT_NY,
                          kind="ExternalInput")
    outx_d = nc.dram_tensor("outx", [2 * 128, SU_pad], F32, kind="ExternalOutput")
    outy_d = nc.dram_tensor("outy", [nplanes * 128, OUTJ], F32,
                            kind="ExternalOutput")

    with tile.TileContext(nc) as tc, ExitStack() as ctx:
        const = ctx.enter_context(tc.tile_pool(name="const", bufs=1))
        act = ctx.enter_context(tc.tile_pool(name="act", bufs=1))
        hpool = ctx.enter_context(tc.tile_pool(name="hpool", bufs=1))
        mlp_ps = ctx.enter_context(tc.tile_pool(name="mlp_ps", bufs=4, space="PSUM"))
        rt_ps = ctx.enter_context(tc.tile_pool(name="rt_ps", bufs=3, space="PSUM"))
        wu_ps = ctx.enter_context(tc.tile_pool(name="wu_ps", bufs=1, space="PSUM"))

        # input DMAs: few coarse dma_starts, ordered by first consumer.
        # Each dma_start costs ~0.8us of serialized trigger time on the
        # sync sequencer, so the emission order IS the stream start order.
        bias_sb = const.tile([128, 20], F32)
        nc.sync.dma_start(bias_sb[:], bias_d[:])
        xt_a_sb = const.tile([128, c_xt_a], DT_A)
        nc.sync.dma_start(xt_a_sb[:], xt_a_d[:])
        w1_tiles = {"y": [None] * KO2, "x": None}
        for lo, hi in w1y_pieces:
            t = const.tile([128, (hi - lo) * c_mo], DT_A, tag=f"w1y{lo}",
                           name=f"w1y{lo}")
            nc.sync.dma_start(t[:], w1y_d[:, lo * c_mo:hi * c_mo])
            v = t.rearrange("p (mo ko m) -> p mo ko m", mo=hi - lo, ko=KO1)
            for mo in range(lo, hi):
                w1_tiles["y"][mo] = v[:, mo - lo]
        if c_xt_b:
            xt_b_sb = const.tile([128, c_xt_b], DT_A)
            nc.sync.dma_start(xt_b_sb[:], xt_b_d[:])
        w2y_sb = const.tile([128, c_w2], DT_A)
        nc.sync.dma_start(w2y_sb[:], w2y_d[:])
        blobx_sb = const.tile([128, c_bx], DT_A)
        nc.sync.dma_start(blobx_sb[:], blobx_d[:])
        w1_tiles["x"] = blobx_sb[:, 0:c_w1].rearrange(
            "p (mo ko m) -> p mo ko m", mo=KO2, ko=KO1)
        w2x_sb = blobx_sb[:, c_w1:c_w1 + c_w2]
        nx_sb = blobx_sb[:, c_w1 + c_w2:]
        ny_sb = []
        for gi in range(NY_GROUPS):
            t = const.tile([128, NY_COLS], DT_NY, tag=f"ny{gi}", name=f"ny{gi}")
            nc.sync.dma_start(t[:], ny_d[gi])
            ny_sb.append(t.rearrange("p (m ko j) -> p m ko j", m=CLS_PER_GROUP, ko=2))

        # PE warmup: tiny dummy matmuls on an uninitialized tile (values are
        # irrelevant, the psum result is never read) keep the PE busy from
        # right after the preamble so HAM un-throttles before real work.
        warm_sb = const.tile([128, 40], BF16)
        nc.any.memset(warm_sb[:], 0)
        wups = wu_ps.tile([40, 40], F32)
        for _ in range(N_WARMUP):
            nc.tensor.matmul(wups[:], lhsT=warm_sb[:, :40], rhs=warm_sb[:],
                             start=True, stop=True)

        xt_vc = [xt_a_sb.rearrange("p (ko b) -> p ko b", ko=KO1)]
        if c_xt_b:
            xt_vc.append(xt_b_sb.rearrange("p (ko b) -> p ko b", ko=KO1))

        def w1v(br, mo, ko, msl):
            if br == "y":
                return w1_tiles["y"][mo][:, ko, msl]
            return w1_tiles["x"][:, mo, ko, msl]

        w2_v = {"y": w2y_sb.rearrange("p (ko m) -> p ko m", ko=KO2),
                "x": w2x_sb.rearrange("p (ko m) -> p ko m", ko=KO2)}
        nx_v = nx_sb.rearrange("p (ko j) -> p ko j", ko=2)
        # bias cols: b1y[0:8] b2y[8:10] b1x[10:18] b2x[18:20]
        bcol = {"y": (0, 8), "x": (10, 18)}

        cls_sb = {}
        CH_MAX = max(c1 - c0 for c0, c1 in chunks)
        for br in ("y", "x"):
            dt_cls = DT_NY if br == "y" else DT_A
            cls_sb[br] = act.tile([128, 2, SU_pad], dt_cls, tag=f"cls{br}",
                                  name=f"cls{br}")
            h_sb = hpool.tile([128, KO2, SU_pad], DT_A, tag="h", name=f"h_{br}")
            b1o, b2o = bcol[br]
            for ci, (c0, c1) in enumerate(chunks):
                cw = c1 - c0
                for mo in range(KO2):
                    ps = mlp_ps.tile([128, CH_MAX], F32, tag="mlp",
                                     name="mlp_ps_t")[:, :cw]
                    for ko in range(KO1):
                        nc.tensor.matmul(
                            ps, lhsT=w1v(br, mo, ko, slice(0, 128)),
                            rhs=xt_vc[ci][:, ko, :cw],
                            start=(ko == 0), stop=(ko == KO1 - 1))
                    nc.scalar.activation(h_sb[:, mo, c0:c1], ps, AFT.Relu,
                                         bias=bias_sb[:, b1o + mo:b1o + mo + 1])
            for c0, c1 in chunks:
                cw = c1 - c0
                for jo in range(2):
                    ps = mlp_ps.tile([128, CH_MAX], F32, tag="mlp",
                                     name="mlp_ps_t")[:, :cw]
                    for ko in range(KO2):
                        nc.tensor.matmul(
                            ps, lhsT=w2_v[br][:, ko, jo * 128:(jo + 1) * 128],
                            rhs=h_sb[:, ko, c0:c1],
                            start=(ko == 0), stop=(ko == KO2 - 1))
                    nc.scalar.activation(cls_sb[br][:, jo, c0:c1], ps, AFT.Identity,
                                         bias=bias_sb[:, b2o + jo:b2o + jo + 1])

        # X output: outX.T[j, b] = sum_i NX[i, j] * clsX.T[i, b]
        outxT = act.tile([128, 2, SU_pad], F32, tag="outxT")
        for c0, c1 in chunks:
            cw = c1 - c0
            for jo in range(2):
                ps = mlp_ps.tile([128, CH_MAX], F32, tag="mlp",
                                 name="mlp_ps_t")[:, :cw]
                for ko in range(2):
                    nc.tensor.matmul(ps, lhsT=nx_v[:, ko, jo * 128:(jo + 1) * 128],
                                     rhs=cls_sb["x"][:, ko, c0:c1],
                                     start=(ko == 0), stop=(ko == 1))
                nc.scalar.copy(outxT[:, jo, c0:c1], ps)
        nc.sync.dma_start(outx_d.rearrange("(jo p) b -> p jo b", p=128), outxT[:])

        # routing: per slot (class m, rows r at compact col s), out_Y.seg =
        # clsY.T[:, s:s+r].T @ NY[m]; psum col-group 32*(k%4), plane k//4
        outy_sb = act.tile([128, nplanes, OUTJ], F32, tag="outy")
        outy_dv = outy_d.rearrange("(o p) j -> p o j", p=128)
        clsy = cls_sb["y"]
        for qi in range(nplanes):
            quad = slots[4 * qi:4 * qi + 4]
            ps = rt_ps.tile([128, OUTJ], F32, tag="rt", name="rt_ps_t")
            for si, (m, _j0, s, r, _k) in enumerate(quad):
                gi, mi = m // CLS_PER_GROUP, m % CLS_PER_GROUP
                for ko in range(2):
                    nc.tensor.matmul(ps[32 * si:32 * si + r, :],
                                     lhsT=clsy[:, ko, s:s + r],
                                     rhs=ny_sb[gi][:, mi, ko, :],
                                     start=(ko == 0), stop=(ko == 1),
                                     tile_position=(0, 32 * si))
            nc.vector.tensor_copy(outy_sb[:, qi, :], ps[:])
            last_of_group = (qi + 1 == nplanes or
                             plane_group[qi + 1] != plane_group[qi])
            if last_of_group:
                p0 = next(i for i in range(nplanes)
                          if plane_group[i] == plane_group[qi])
                nc.sync.dma_start(outy_dv[:, p0:qi + 1, :],
                                  outy_sb[:, p0:qi + 1, :])

    nc.compile()
    _NC_CACHE[key] = nc
    return nc


def _prepare_inputs(plan, state, option, embed_table, Wx1, bx1, Wx2, bx2,
                    Wy1, by1, Wy2, by2, noise_lib_X, noise_lib_Y):
    np_a = _NP_MAP[DT_A_NAME]
    np_ny = _NP_MAP[DT_NY_NAME]
    SU_pad = plan["SU_pad"]
    opt = plan["opt"]
    core_of, col_of = plan["core_of"], plan["col_of"]

    state = np.asarray(state, np.float32)
    embed_table = np.asarray(embed_table, np.float32)

    # per-core feature-major inputs
    Xall = np.zeros((NCORES, SU_pad, D_PAD), np.float32)
    Xall[core_of, col_of, :FEAT] = state
    Xall[core_of, col_of, FEAT:D_IN] = embed_table[opt]
    # [NCORES, 128, KO1, SU_pad]
    xt = Xall.transpose(0, 2, 1).reshape(NCORES, KO1, 128, SU_pad) \
        .transpose(0, 2, 1, 3).astype(np_a)
    ch0 = plan["chunks"][0][1]
    xt_a = np.ascontiguousarray(xt[:, :, :, :ch0]).reshape(NCORES, 128, -1)
    xt_b = np.ascontiguousarray(xt[:, :, :, ch0:]).reshape(NCORES, 128, -1)

    def pack_w1(w):
        # mo-major: [128p, mo, ko, 128] flattened
        wp = np.zeros((D_PAD, HID), np.float32)
        wp[:D_IN] = np.asarray(w, np.float32)
        return wp.reshape(KO1, 128, KO2, 128).transpose(1, 2, 0, 3) \
            .reshape(128, KO1 * HID)

    def pack_w2(w):
        return np.asarray(w, np.float32).reshape(KO2, 128, LIB) \
            .transpose(1, 0, 2).reshape(128, KO2 * LIB)

    nx = np.asarray(noise_lib_X, np.float32).reshape(2, 128, OUTJ) \
        .transpose(1, 0, 2).reshape(128, 2 * OUTJ)
    w1y = np.ascontiguousarray(pack_w1(Wy1).astype(np_a))
    w2y = np.ascontiguousarray(pack_w2(Wy2).astype(np_a))
    blobx = np.ascontiguousarray(np.concatenate(
        [pack_w1(Wx1), pack_w2(Wx2), nx], axis=1).astype(np_a))

    bias = np.zeros((128, 20), np.float32)
    bias[:, 0:8] = np.asarray(by1, np.float32).reshape(8, 128).T
    bias[:, 8:10] = np.asarray(by2, np.float32).reshape(2, 128).T
    bias[:, 10:18] = np.asarray(bx1, np.float32).reshape(8, 128).T
    bias[:, 18:20] = np.asarray(bx2, np.float32).reshape(2, 128).T

    ny = np.ascontiguousarray(
        np.asarray(noise_lib_Y, np.float32)
        .reshape(NY_GROUPS, CLS_PER_GROUP, 2, 128, OUTJ)
        .transpose(0, 3, 1, 2, 4)
        .reshape(NY_GROUPS, 128, CLS_PER_GROUP * 2 * OUTJ).astype(np_ny))

    in_maps = []
    for c in range(NCORES):
        m = {"xt_a": xt_a[c], "w1y": w1y, "w2y": w2y,
             "blobx": blobx, "bias": bias, "ny": ny}
        if xt_b.shape[-1]:
            m["xt_b"] = xt_b[c]
        in_maps.append(m)
    return in_maps


def _gather_outputs(plan, results):
    core_of, col_of, outrow_of = (plan["core_of"], plan["col_of"],
                                  plan["outrow_of"])
    ox = np.stack([r["outx"] for r in results])   # [8, 256, SU_pad]
    oy = np.stack([r["outy"] for r in results])   # [8, nplanes*128, OUTJ]
    gx = ox.transpose(0, 2, 1)[core_of, col_of].astype(np.float32)
    gy = oy[core_of, outrow_of].astype(np.float32)
    return gx, gy


def _run(inputs, trace=False):
    plan = _plan(inputs["option"])
    nc = _build_nc(plan)
    in_maps = _prepare_inputs(plan, **inputs)
    res = run_bass_kernel_spmd(nc, in_maps, core_ids=list(range(NCORES)),
                               trace=trace)
    gx, gy = _gather_outputs(plan, res.results)
    return (gx, gy), res


def kernel(**inputs):
    (gx, gy), _ = _run(inputs, trace=False)
    return gx, gy
